# revision 33
# baseline (speedup 1.0000x reference)
"""Soft-MoE forward on 8 TRN2 NeuronCores — v7 (TimelineSim-guided pipelining).

Data-parallel over batch (B=16 -> 2 per core). Matmuls in bf16 except the
combine, which runs fp8e4 DoubleRow (half the instructions at 2x rate).
Full fp8 breaks the 2e-2 accuracy gate (quantization noise ~3.6%/tensor
does not average through random-sign contractions; measured 6.8e-2
all-fp8, 3.5e-2 for router/dispatch-fp8) but combine-only fp8 lands at
1.36e-2: PT is cast bf16->fp8 for free in the gpsimd store DMA, and eo is
scaled by 2^13 into fp8 range (descale folded into the rzc factor).

v8 structural changes vs v2 (each validated against concourse TimelineSim;
single-rep sim 1596us -> 1323us, steady-rep 1298us; HW-validated
rel err 1.363581e-2, chain-slope 1.665ms/rep):
- P split into two 32KB arenas (slot halves A/B); eo is one [P,BL,St,C]
  fp8 tile reusing arena A's slot. Arena B and the wrt arena alternate
  roles per rep ("bigX"/"bigZ") so the NEXT rep's router weights (8MB)
  prefetch during the expert phase instead of stalling the rep boundary.
- w_proj streams as 8 half-chunks/expert (bufs=7) issued from inside the
  PREVIOUS expert's eo — the window where the DMA pipe is otherwise
  idle — so they neither stall eo nor crowd out wg/wf chunk loads.
- wg/wf stream in [P,Ct,256] chunks (8/expert, bufs=3) on the sync ring
  only: DMA doorbells that wait on buffer slots must not share a
  sequencer with compute (ACT) — head-of-line blocking of the silu
  delays PSUM release and stalls PE.
- ht bufs=4 for the same reason: the h-transpose doorbell on the ACT
  ring must never wait on a slot.
- Router t=0/t=1 of the first batch accumulate c-chunk-major across all
  8 PSUM banks so PE works while the 8MB wrt load is still landing.
- First 5 dispatch xk loads pre-issued at router t==1 (the list
  scheduler otherwise parks them behind the next batch's router loads,
  which head-of-line block the SP ring for ~7us).
- Dispatch PSUM tags flipped (pb first): pa banks still drain the last
  router exp() when dispatch starts.
- y stores on the scalar HWDGE ring (gpsimd Q7 backs up under 64
  stores); last tiles split across both rings to shorten the drain.
- zd accumulated with running DVE adds (no [P,St,Tt] zdall buffer).

DO NOT change dynamic_dma_scratch_size: a non-default SWDGE carveout
(12288) produced NaN output on hardware (descriptor-ring corruption).
"""

import numpy as np
import ml_dtypes

import concourse.bass as bass
import concourse.tile as tile
from concourse import mybir

B, T, C, E, H = 16, 2048, 1024, 16, 2048
CAP = T // E  # 128
S = E * CAP  # 2048 slots
P = 128
NCORES = 8
BL = B // NCORES  # 2

Tt = T // P   # 16
Ct = C // P   # 8
St = S // P   # 16
Ht = H // P   # 16

FP32 = mybir.dt.float32
BF16 = mybir.dt.bfloat16
F8E4 = mybir.dt.float8e4
DR = mybir.MatmulPerfMode.DoubleRow
EOSC = 2.0 ** 13  # eo fp8 storage scale; folded out via rzc
AX = mybir.AluOpType
AF = mybir.ActivationFunctionType


def _split_multi_waits(nc):
    """This walrus build accepts only ONE sync wait per instruction; Tile's
    wait-assignment can emit several. Move extra waits onto single-wait nops
    inserted just before the instruction on the same engine."""
    import bass_rust

    nid = 0
    for f in nc.m.functions:
        for bb in f.blocks:
            out = []
            changed = False
            for inst in bb.instructions:
                si = inst.sync_info
                waits = list(si.on_wait) if si and si.on_wait else []
                if len(waits) > 1:
                    changed = True
                    for w in waits[:-1]:
                        nop = mybir.InstNoOp(name=f"TW-{nid}", ins=[], outs=[])
                        nid += 1
                        nop.engine = inst.engine
                        nop.sync_info = bass_rust.SyncInfo(on_wait=[w], on_update=[])
                        out.append(nop)
                    si.on_wait = waits[-1:]
                out.append(inst)
            if changed:
                bb.instructions = out


def build_nc(loops=BL, split_waits=True):
    assert loops % BL == 0
    nreps = loops // BL
    nc = bass.Bass(trn_type="TRN2")

    xb = nc.dram_tensor("xb", [BL, T, C], BF16, kind="ExternalInput")
    xbt = nc.dram_tensor("xbt", [BL, C, T], BF16, kind="ExternalInput")
    wrt = nc.dram_tensor("wrt", [C, S], BF16, kind="ExternalInput")
    wg = nc.dram_tensor("wg", [E, C, H], BF16, kind="ExternalInput")
    wf = nc.dram_tensor("wf", [E, C, H], BF16, kind="ExternalInput")
    wp = nc.dram_tensor("wp", [E, H, C], BF16, kind="ExternalInput")
    y = nc.dram_tensor("y", [BL, T, C], FP32, kind="ExternalOutput")

    HQ = 256  # wg/wf h-chunk width

    with tile.TileContext(nc) as tc:
        with (
            tc.tile_pool(name="dram", bufs=2, space="DRAM") as dpool,
            tc.tile_pool(name="big", bufs=1) as cpool,
            tc.tile_pool(name="st3", bufs=3) as p3,
            tc.tile_pool(name="st2", bufs=2) as p2,
            tc.tile_pool(name="stat", bufs=2) as sp,
            tc.tile_pool(name="psum", bufs=4, space="PSUM") as pp,
        ):
            # round-robin PSUM->SBUF copy engine to avoid single-queue tails
            def _copy(i, out, in_):
                # gpsimd cannot read PSUM (walrus birverifier)
                if i % 2 == 0:
                    nc.scalar.copy(out, in_)
                else:
                    nc.vector.tensor_copy(out, in_)

            def emit_wrt_chunk(wtile, c):
                eng = nc.scalar if c % 2 == 0 else nc.gpsimd
                eng.dma_start(
                    wtile[:, c], wrt.rearrange("(c p) s -> p c s", p=P)[:, c]
                )

            # rep-parity arena roles: "bigY" always holds P-half-A then eo;
            # "bigX"/"bigZ" alternate between wrt and P-half-B so the next
            # rep's wrt prefetches into the arena freed by this rep's P-B.
            wrt_next = cpool.tile([P, Ct, S], BF16, tag="bigX", bufs=1,
                                  name="wrt_sb")
            for c in range(Ct):
                emit_wrt_chunk(wrt_next, c)

            for rep in range(nreps):
                wrt_sb = wrt_next
                btag = "bigZ" if rep % 2 == 0 else "bigX"

                pt_drams, rzcs, rzds = {}, {}, {}
                rzc2s = {}
                eit_drams = {}
                p_as, p_bs = {}, {}
                pre_e0 = {}
                for b in range(BL):
                    # ---- R+T: router logits, exp, transpose chunks ----
                    p_a = cpool.tile([P, Tt, S // 2], BF16, tag="bigY", bufs=1,
                                     name=f"p_a{b}")
                    p_b = cpool.tile([P, Tt, S // 2], BF16, tag=btag, bufs=1,
                                     name=f"p_b{b}")
                    p_as[b], p_bs[b] = p_a, p_b
                    pt_dram = dpool.tile([Tt, P, S], F8E4, tag="pt_dram",
                                         name=f"ptd{b}")
                    eit_dram = dpool.tile([P, Ct, S], BF16, tag="eit_dram",
                                          name=f"eitd{b}")
                    pt_drams[b], eit_drams[b] = pt_dram, eit_dram
                    rzc = sp.tile([P, Tt], FP32, tag="rzc", name=f"rzc{b}")
                    rzd = sp.tile([P, St], FP32, tag="rzd", name=f"rzd{b}")
                    rzcs[b], rzds[b] = rzc, rzd
                    zd = sp.tile([P, St], FP32, tag="zd", bufs=2, name="zd")

                    # pre-issued first dispatch xk loads (filled at t==1: at
                    # t==0 they'd push the first router loads off the pipe)
                    xk_pre = []

                    def emit_xk_pre():
                        for k in range(5):
                            xk = p3.tile([P, 512], BF16, tag="xk", bufs=5,
                                         name="xk")
                            nc.sync.dma_start(
                                xk[:],
                                xb[b].rearrange("(k p) c -> p k c", p=P)[
                                    :, k, 0:512
                                ],
                            )
                            xk_pre.append(xk)

                    def load_xbt(t):
                        xbt_t = p3.tile([P, Ct, P], BF16, tag="xbt_t", bufs=2,
                                        name="xbt_t")
                        nc.sync.dma_start(
                            xbt_t[:],
                            xbt[b].rearrange("(c p) t -> p c t", p=P)[
                                :, :, t * P : (t + 1) * P
                            ],
                        )
                        return xbt_t

                    def router_tail(t, gps):
                        zc4 = sp.tile([P, 4], FP32, tag="zc4", bufs=2, name="zc4")
                        for n in range(4):
                            dst = (
                                p_a[:, t, n * 512 : (n + 1) * 512]
                                if n < 2
                                else p_b[:, t, (n - 2) * 512 : (n - 1) * 512]
                            )
                            nc.scalar.activation(
                                dst,
                                gps[n][:],
                                AF.Exp,
                                accum_out=zc4[:, n : n + 1],
                            )
                        zc1 = sp.tile([P, 1], FP32, tag="zc1", bufs=2, name="zc1")
                        nc.vector.tensor_reduce(zc1[:], zc4[:], mybir.AxisListType.X, AX.add)
                        nc.vector.reciprocal(rzc[:, t : t + 1], zc1[:])

                        # transpose chunk t in two half-slabs (smaller
                        # staging; DMA engines overlap PE on t+1)
                        for sh in range(2):
                            src = p_a if sh == 0 else p_b
                            ptc = p3.tile([P, St // 2, P], BF16, tag="ptc",
                                          bufs=2, name="ptc")
                            nc.scalar.dma_start_transpose(
                                ptc[:], src[:, t, :]
                            )
                            zds = zd[:, sh * 8 : (sh + 1) * 8]
                            if t == 0:
                                nc.vector.tensor_reduce(
                                    zds, ptc[:], mybir.AxisListType.X, AX.add
                                )
                            else:
                                zdt = sp.tile([P, St // 2], FP32, tag="zdt",
                                              bufs=2, name="zdt")
                                nc.vector.tensor_reduce(
                                    zdt[:], ptc[:], mybir.AxisListType.X, AX.add
                                )
                                nc.vector.tensor_add(zds, zds, zdt[:])
                            nc.gpsimd.dma_start(
                                pt_dram[t, :, sh * 1024 : (sh + 1) * 1024],
                                ptc[:],
                            )

                    tstart = 0
                    if rep == 0 and b == 0:
                        # wrt is still streaming in from HBM: accumulate the
                        # first two token-tiles chunk-by-chunk across all 8
                        # PSUM banks so PE works while the 8MB load lands
                        tstart = 2
                        xbt01 = [load_xbt(0), load_xbt(1)]
                        emit_xk_pre()
                        gps01 = [
                            [pp.tile([P, 512], FP32,
                                     tag=("pa" if tt == 0 else "pb"),
                                     name=f"gps{tt}_{n}") for n in range(4)]
                            for tt in range(2)
                        ]
                        for c in range(Ct):
                            for tt in range(2):
                                for n in range(4):
                                    nc.tensor.matmul(
                                        gps01[tt][n][:],
                                        xbt01[tt][:, c, :],
                                        wrt_sb[:, c, n * 512 : (n + 1) * 512],
                                        start=(c == 0),
                                        stop=(c == Ct - 1),
                                    )
                        router_tail(0, gps01[0])
                        router_tail(1, gps01[1])

                    for t in range(tstart, Tt):
                        if t == 1:
                            emit_xk_pre()
                        xbt_t = load_xbt(t)
                        gps = [pp.tile([P, 512], FP32, tag="pa", name=f"gps{n}")
                               for n in range(4)]
                        for c in range(Ct):
                            for n in range(4):
                                nc.tensor.matmul(
                                    gps[n][:],
                                    xbt_t[:, c, :],
                                    wrt_sb[:, c, n * 512 : (n + 1) * 512],
                                    start=(c == 0),
                                    stop=(c == Ct - 1),
                                )
                        router_tail(t, gps)

                    nc.vector.reciprocal(rzd[:], zd[:])
                    rzc2 = sp.tile([P, Tt], FP32, tag="rzc2", name=f"rzc2{b}")
                    nc.vector.tensor_scalar_mul(rzc2[:], rzc[:], 1.0 / EOSC)
                    rzc2s[b] = rzc2

                    # ---- D: dispatch eit = x^T @ P (unnormalized) ----
                    # nh outer: slot-halves complete in order, so expert 0's
                    # loads (prefetched below) start while nh=1 still runs.
                    for nh in range(2):
                        p_sb = p_as[b] if nh == 0 else p_bs[b]
                        if b == 1 and nh == 1:
                            pre_e0["eite"] = {}
                            for bb in range(BL):
                                eit_e = p2.tile([P, Ct, P], BF16, tag="eit_e",
                                                bufs=3, name=f"eite{bb}")
                                nc.sync.dma_start(
                                    eit_e[:],
                                    eit_drams[bb][:, :, 0:P],
                                )
                                pre_e0["eite"][bb] = eit_e
                            wgc = p2.tile([P, Ct, HQ], BF16, tag="wgc",
                                          bufs=4, name="wgc")
                            nc.sync.dma_start(
                                wgc[:],
                                wg[0].rearrange("(c p) h -> p c h", p=P)[:, :, 0:HQ],
                            )
                            wfcc = p2.tile([P, Ct, HQ], BF16, tag="wfcc",
                                           bufs=4, name="wfcc")
                            nc.gpsimd.dma_start(
                                wfcc[:],
                                wf[0].rearrange("(c p) h -> p c h", p=P)[:, :, 0:HQ],
                            )
                            pre_e0["w0"] = (wgc, wfcc)
                        for mh in range(2):
                            # pb first: pa banks are still draining the last
                            # router exp() when dispatch starts
                            dps = [pp.tile([P, 512], FP32,
                                           tag=("pb" if i < 4 else "pa"),
                                           name=f"dps{i}")
                                   for i in range(8)]
                            for k in range(Tt):
                                if nh == 0 and mh == 0 and k < 5:
                                    xk = xk_pre[k]
                                else:
                                    xk = p3.tile([P, 512], BF16, tag="xk",
                                                 bufs=5, name="xk")
                                    nc.sync.dma_start(
                                        xk[:],
                                        xb[b].rearrange(
                                            "(k p) c -> p k c", p=P
                                        )[:, k, mh * 512 : (mh + 1) * 512],
                                    )
                                for m4 in range(4):
                                    for n2 in range(2):
                                        nc.tensor.matmul(
                                            dps[m4 * 2 + n2][:],
                                            xk[:, m4 * P : (m4 + 1) * P],
                                            p_sb[:, k,
                                                 n2 * 512 : (n2 + 1) * 512],
                                            start=(k == 0),
                                            stop=(k == Tt - 1),
                                        )
                            for i in range(8):
                                m = mh * 4 + i // 2
                                n = nh * 2 + i % 2
                                est = p3.tile([P, 512], BF16, tag="est", bufs=3,
                                              name="est")
                                _copy(i, est[:], dps[i][:])
                                nc.gpsimd.dma_start(
                                    eit_dram[:, m, n * 512 : (n + 1) * 512],
                                    est[:],
                                )

                # ---- M: per-expert GLU MLP, software-pipelined: expert e's
                # gg/hh runs on PE while expert e-1's h transposes on the DMA
                # engines; e-1's eo matmuls then fill what would be the
                # transpose stall. eo -> one fp8 tile resident in P-A's arena.
                eo_t = cpool.tile([P, BL, St, C], F8E4, tag="bigY", bufs=1,
                                  name="eo_t")
                if rep + 1 < nreps:
                    wrt_next = cpool.tile([P, Ct, S], BF16, tag=btag, bufs=1,
                                          name="wrt_sb")
                else:
                    wrt_next = None

                def emit_prefetch(e, into):
                    into["eite"] = {}

                    def _eite(bb):
                        eit_e = p2.tile([P, Ct, P], BF16, tag="eit_e",
                                        bufs=3, name=f"eite{bb}")
                        nc.sync.dma_start(
                            eit_e[:],
                            eit_drams[bb][:, :, e * P : (e + 1) * P],
                        )
                        into["eite"][bb] = eit_e

                    _eite(0)
                    wgc = p2.tile([P, Ct, HQ], BF16, tag="wgc", bufs=4,
                                  name="wgc")
                    nc.sync.dma_start(
                        wgc[:],
                        wg[e].rearrange("(c p) h -> p c h", p=P)[:, :, 0:HQ],
                    )
                    wfcc = p2.tile([P, Ct, HQ], BF16, tag="wfcc", bufs=4,
                                   name="wfcc")
                    nc.sync.dma_start(
                        wfcc[:],
                        wf[e].rearrange("(c p) h -> p c h", p=P)[:, :, 0:HQ],
                    )
                    _eite(1)
                    into["w0"] = (wgc, wfcc)

                def issue_wpc_chunk(e, kh):
                    # w_proj half-chunk (2 k-tiles) for expert e's eo
                    w = p2.tile([P, 2, C], BF16, tag="wpc", bufs=5,
                                name="wpc")
                    nc.gpsimd.dma_start(
                        w[:],
                        wp[e].rearrange("(k p) c -> p k c", p=P)[
                            :, kh * 2 : (kh + 1) * 2, :
                        ],
                    )
                    return w

                def issue_wpc(e):
                    return [issue_wpc_chunk(e, kh) for kh in range(8)]

                def emit_gghh(e, pre):
                    if wrt_next is not None and 3 <= e < 3 + Ct:
                        emit_wrt_chunk(wrt_next, e - 3)
                    eites = pre["eite"]
                    hs = {
                        b: p2.tile([P, H], BF16, tag=f"hs{b}", bufs=1, name=f"hsb{b}")
                        for b in range(BL)
                    }
                    for hc in range(H // HQ):
                        if hc == 0:
                            wgc, wfcc = pre["w0"]
                        else:
                            wgc = p2.tile([P, Ct, HQ], BF16, tag="wgc",
                                          bufs=4, name="wgc")
                            nc.sync.dma_start(
                                wgc[:],
                                wg[e].rearrange("(c p) h -> p c h", p=P)[
                                    :, :, hc * HQ : (hc + 1) * HQ
                                ],
                            )
                            wfcc = p2.tile([P, Ct, HQ], BF16, tag="wfcc",
                                           bufs=4, name="wfcc")
                            nc.sync.dma_start(
                                wfcc[:],
                                wf[e].rearrange("(c p) h -> p c h", p=P)[
                                    :, :, hc * HQ : (hc + 1) * HQ
                                ],
                            )
                        for b in range(BL):
                            gg = pp.tile([P, HQ], FP32, tag="pa", name=f"gg{b}")
                            hh = pp.tile([P, HQ], FP32, tag="pa", name=f"hh{b}")
                            for c in range(Ct):
                                # same stationary operand back-to-back: lets
                                # codegen/hw skip the redundant LDWEIGHTS
                                nc.tensor.matmul(
                                    gg[:], eites[b][:, c, :], wgc[:, c, :],
                                    start=(c == 0), stop=(c == Ct - 1),
                                )
                                nc.tensor.matmul(
                                    hh[:], eites[b][:, c, :], wfcc[:, c, :],
                                    start=(c == 0), stop=(c == Ct - 1),
                                )
                            sg = p3.tile([P, HQ], BF16, tag="sg", bufs=1,
                                         name="sg")
                            nc.scalar.activation(
                                sg[:], gg[:], AF.Silu,
                                scale=rzds[b][:, e : e + 1],
                            )
                            nc.vector.scalar_tensor_tensor(
                                hs[b][:, hc * HQ : (hc + 1) * HQ],
                                hh[:], rzds[b][:, e : e + 1], sg[:],
                                AX.mult, AX.mult,
                            )
                    hts = {}
                    for b in range(BL):
                        ht = p2.tile([P, Ht, P], BF16, tag="ht", bufs=4, name=f"htb{b}")
                        nc.scalar.dma_start_transpose(ht[:], hs[b][:])
                        hts[b] = ht
                    return hts

                def emit_eo(e, hts, wpcs, wpc_pre_e=None, wpcs_out=None):
                    # cc innermost: both C-halves reuse the same stationary
                    # ht chunk (halves the LDWEIGHTS count). The next
                    # expert's w_proj chunks are issued here — eo's window
                    # is when the DMA pipe is otherwise idle.
                    eops = {
                        b: [pp.tile([P, 512], FP32, tag="pb",
                                    name=f"eop{b}_{cc}") for cc in range(2)]
                        for b in range(BL)
                    }
                    for kh in range(8):
                        if wpc_pre_e is not None:
                            wpcs_out[wpc_pre_e].append(
                                issue_wpc_chunk(wpc_pre_e, kh)
                            )
                        wpc = wpcs[kh]
                        for b in range(BL):
                            for k in range(2):
                                for cc in range(2):
                                    nc.tensor.matmul(
                                        eops[b][cc][:],
                                        hts[b][:, kh * 2 + k, :],
                                        wpc[:, k, cc * 512 : (cc + 1) * 512],
                                        start=(kh == 0 and k == 0),
                                        stop=(kh == 7 and k == 1),
                                    )
                    for b in range(BL):
                        for cc in range(2):
                            nc.vector.tensor_scalar_mul(
                                eo_t[:, b, e, cc * 512 : (cc + 1) * 512],
                                eops[b][cc][:], EOSC,
                            )

                prev = None
                pre = pre_e0
                wpcs_store = {}
                for e in range(E):
                    hts, pre_next = emit_gghh(e, pre), {}
                    if e == 0:
                        wpcs_store[0] = issue_wpc(0)
                    if e + 1 < E:
                        emit_prefetch(e + 1, pre_next)
                    if prev is not None:
                        nxt = e if e < E else None
                        wpcs_store[nxt] = []
                        emit_eo(prev[0], prev[1], wpcs_store.pop(prev[0]),
                                wpc_pre_e=nxt, wpcs_out=wpcs_store)
                    prev = (e, hts)
                    pre = pre_next
                emit_eo(prev[0], prev[1], wpcs_store.pop(prev[0]))

                # ---- C: combine y = (P^T^T @ eo) * rzc ----
                for b in range(BL):
                    for t in range(Tt):
                        ptr = p3.tile([P, St, P], F8E4, tag="ptr", bufs=3,
                                      name="ptr")
                        nc.sync.dma_start(ptr[:], pt_drams[b][t])
                        ypss = [pp.tile([P, 512], FP32,
                                        tag=("pa" if cc == 0 else "pb"),
                                        name=f"yps{cc}") for cc in range(2)]
                        for e in range(0, St, 2):
                            for cc in range(2):
                                nc.tensor.matmul(
                                    ypss[cc][:],
                                    ptr[:, e : e + 2, :],
                                    eo_t[:, b, e : e + 2,
                                         cc * 512 : (cc + 1) * 512],
                                    start=(e == 0),
                                    stop=(e == St - 2),
                                    perf_mode=DR,
                                )
                        for cc in range(2):
                            ysb = p3.tile([P, 512], FP32, tag="ysb", bufs=3,
                                          name="ysb")
                            nc.vector.tensor_scalar_mul(
                                ysb[:], ypss[cc][:], rzc2s[b][:, t : t + 1]
                            )
                            # y stores split across both HWDGE rings: the
                            # store-completion -> ysb-slot -> DVE-scale ->
                            # PSUM-release chain paces the combine, and one
                            # ring's completion latency is the bottleneck.
                            # (gpsimd's Q7 would back up under 64 stores.)
                            eng = nc.scalar if cc == 0 else nc.sync
                            eng.dma_start(
                                y[b, t * P : (t + 1) * P,
                                  cc * 512 : (cc + 1) * 512],
                                ysb[:],
                            )
    if split_waits:
        _split_multi_waits(nc)
    return nc


def make_in_maps(x, w_router_gate, w_fc, w_gate, w_proj):
    bf16 = ml_dtypes.bfloat16
    wrt_np = np.ascontiguousarray(w_router_gate.reshape(S, C).T).astype(bf16)
    wg_np = w_gate.astype(bf16)
    wf_np = w_fc.astype(bf16)
    wp_np = w_proj.astype(bf16)

    in_maps = []
    for c in range(NCORES):
        xc = x[c * BL : (c + 1) * BL]
        xb_np = xc.astype(bf16)
        xbt_np = np.ascontiguousarray(xb_np.transpose(0, 2, 1))
        in_maps.append(
            {"xb": xb_np, "xbt": xbt_np, "wrt": wrt_np,
             "wg": wg_np, "wf": wf_np, "wp": wp_np}
        )
    return in_maps


def kernel(x, w_router_gate, w_fc, w_gate, w_proj):
    in_maps = make_in_maps(x, w_router_gate, w_fc, w_gate, w_proj)

    from concourse.bass_utils import run_bass_kernel_spmd

    nc = build_nc()
    res = None
    last_err = None
    for attempt in range(4):
        try:
            res = run_bass_kernel_spmd(nc, in_maps, core_ids=list(range(NCORES)))
            break
        except Exception as e:  # transient NRT_EXEC_UNIT_UNRECOVERABLE on first exec
            last_err = e
            import time as _time

            _time.sleep(5)
    if res is None:
        raise last_err
    y = np.concatenate(
        [res.results[c]["y"] for c in range(NCORES)], axis=0
    ).astype(np.float32)
    return y


if __name__ == "__main__":
    print("built", build_nc())


# revision 46
# speedup vs baseline: 1.0752x; 1.0752x over previous
"""Soft-MoE forward on 8 TRN2 NeuronCores — v7 (TimelineSim-guided pipelining).

Data-parallel over batch (B=16 -> 2 per core). Matmuls in bf16 except the
combine, which runs fp8e4 DoubleRow (half the instructions at 2x rate).
Full fp8 breaks the 2e-2 accuracy gate (quantization noise ~3.6%/tensor
does not average through random-sign contractions; measured 6.8e-2
all-fp8, 3.5e-2 for router/dispatch-fp8) but combine-only fp8 lands at
1.36e-2: PT is cast bf16->fp8 for free in the gpsimd store DMA, and eo is
scaled by 2^13 into fp8 range (descale folded into the rzc factor).

v9 structural changes vs v2 (each validated against concourse TimelineSim;
single-rep sim 1596us -> 1308us, steady-rep 1283us; HW-validated
rel err 1.363581e-2):
- P split into two 32KB arenas (slot halves A/B); eo is one [P,BL,St,C]
  fp8 tile reusing arena A's slot. Arena B and the wrt arena alternate
  roles per rep ("bigX"/"bigZ") so the NEXT rep's router weights (8MB)
  prefetch during the expert phase instead of stalling the rep boundary.
- w_proj streams as 8 half-chunks/expert (bufs=5) issued from inside the
  PREVIOUS expert's eo — the window where the DMA pipe is otherwise
  idle — so they neither stall eo nor crowd out wg/wf chunk loads.
- wg/wf stream in [P,Ct,256] chunks (8/expert, bufs=4) on the sync ring
  only: DMA doorbells that wait on buffer slots must not share a
  sequencer with compute (ACT) — head-of-line blocking of the silu
  delays PSUM release and stalls PE. bufs=4 (not 3) was the binding
  constraint for gghh streaming (-49us).
- fp8 limits (numpy-emulated, calibrates to HW within 0.02e-2): w_proj
  fp8 = 3.2e-2 (fails), h-only fp8 = 1.62e-2 (passes but matmul
  operands must share dtype, so no eo DoubleRow). eo stays bf16.
- ht bufs=4 for the same reason: the h-transpose doorbell on the ACT
  ring must never wait on a slot.
- Router t=0/t=1 of the first batch accumulate c-chunk-major across all
  8 PSUM banks so PE works while the 8MB wrt load is still landing.
- First 5 dispatch xk loads pre-issued at router t==1 (the list
  scheduler otherwise parks them behind the next batch's router loads,
  which head-of-line block the SP ring for ~7us).
- Dispatch PSUM tags flipped (pb first): pa banks still drain the last
  router exp() when dispatch starts.
- y stores on the scalar HWDGE ring (gpsimd Q7 backs up under 64
  stores); last tiles split across both rings to shorten the drain.
- zd accumulated with running DVE adds (no [P,St,Tt] zdall buffer).
- Phase-disjoint tag sharing: tiles of different shapes may share a tag
  (serial slot reuse, deps auto-inserted). Combine's ysb staging lives
  in wgc's arena (idle during combine), dispatch's est in wfcc's (idle
  during dispatch), startup xbt t=2/3 in ht's (idle at rep start) —
  freeing ~9KB of dedicated arenas to fund xk=6/eite=4/xbt_t=3 depth.

DO NOT change dynamic_dma_scratch_size: a non-default SWDGE carveout
(12288) produced NaN output on hardware (descriptor-ring corruption).
"""

import numpy as np
import ml_dtypes

import concourse.bass as bass
import concourse.tile as tile
from concourse import mybir

B, T, C, E, H = 16, 2048, 1024, 16, 2048
CAP = T // E  # 128
S = E * CAP  # 2048 slots
P = 128
NCORES = 8
BL = B // NCORES  # 2

Tt = T // P   # 16
Ct = C // P   # 8
St = S // P   # 16
Ht = H // P   # 16

FP32 = mybir.dt.float32
BF16 = mybir.dt.bfloat16
F8E4 = mybir.dt.float8e4
DR = mybir.MatmulPerfMode.DoubleRow
EOSC = 2.0 ** 13  # eo fp8 storage scale; folded out via rzc
AX = mybir.AluOpType
AF = mybir.ActivationFunctionType


def _split_multi_waits(nc):
    """This walrus build accepts only ONE sync wait per instruction; Tile's
    wait-assignment can emit several. Move extra waits onto single-wait nops
    inserted just before the instruction on the same engine."""
    import bass_rust

    nid = 0
    for f in nc.m.functions:
        for bb in f.blocks:
            out = []
            changed = False
            for inst in bb.instructions:
                si = inst.sync_info
                waits = list(si.on_wait) if si and si.on_wait else []
                if len(waits) > 1:
                    changed = True
                    for w in waits[:-1]:
                        nop = mybir.InstNoOp(name=f"TW-{nid}", ins=[], outs=[])
                        nid += 1
                        nop.engine = inst.engine
                        nop.sync_info = bass_rust.SyncInfo(on_wait=[w], on_update=[])
                        out.append(nop)
                    si.on_wait = waits[-1:]
                out.append(inst)
            if changed:
                bb.instructions = out


def build_nc(loops=BL, split_waits=True):
    assert loops % BL == 0
    nreps = loops // BL
    nc = bass.Bass(trn_type="TRN2")

    xb = nc.dram_tensor("xb", [BL, T, C], BF16, kind="ExternalInput")
    xbt = nc.dram_tensor("xbt", [BL, C, T], BF16, kind="ExternalInput")
    wrt = nc.dram_tensor("wrt", [C, S], BF16, kind="ExternalInput")
    wg = nc.dram_tensor("wg", [E, C, H], BF16, kind="ExternalInput")
    wf = nc.dram_tensor("wf", [E, C, H], BF16, kind="ExternalInput")
    wp = nc.dram_tensor("wp", [E, H, C], BF16, kind="ExternalInput")
    y = nc.dram_tensor("y", [BL, T, C], FP32, kind="ExternalOutput")

    HQ = 256  # wg/wf h-chunk width

    with tile.TileContext(nc) as tc:
        with (
            tc.tile_pool(name="dram", bufs=2, space="DRAM") as dpool,
            tc.tile_pool(name="big", bufs=1) as cpool,
            tc.tile_pool(name="st3", bufs=3) as p3,
            tc.tile_pool(name="st2", bufs=2) as p2,
            tc.tile_pool(name="stat", bufs=2) as sp,
            tc.tile_pool(name="psum", bufs=4, space="PSUM") as pp,
        ):
            # round-robin PSUM->SBUF copy engine to avoid single-queue tails
            def _copy(i, out, in_):
                # gpsimd cannot read PSUM (walrus birverifier)
                if i % 2 == 0:
                    nc.scalar.copy(out, in_)
                else:
                    nc.vector.tensor_copy(out, in_)

            def emit_wrt_chunk(wtile, c):
                eng = nc.scalar if c % 2 == 0 else nc.gpsimd
                eng.dma_start(
                    wtile[:, c], wrt.rearrange("(c p) s -> p c s", p=P)[:, c]
                )

            # rep-parity arena roles: "bigY" always holds P-half-A then eo;
            # "bigX"/"bigZ" alternate between wrt and P-half-B so the next
            # rep's wrt prefetches into the arena freed by this rep's P-B.
            wrt_next = cpool.tile([P, Ct, S], BF16, tag="bigX", bufs=1,
                                  name="wrt_sb")
            for c in range(Ct):
                emit_wrt_chunk(wrt_next, c)

            for rep in range(nreps):
                wrt_sb = wrt_next
                btag = "bigZ" if rep % 2 == 0 else "bigX"

                pt_drams, rzcs, rzds = {}, {}, {}
                rzc2s = {}
                eit_drams = {}
                p_as, p_bs = {}, {}
                pre_e0 = {}
                for b in range(BL):
                    # ---- R+T: router logits, exp, transpose chunks ----
                    p_a = cpool.tile([P, Tt, S // 2], BF16, tag="bigY", bufs=1,
                                     name=f"p_a{b}")
                    p_b = cpool.tile([P, Tt, S // 2], BF16, tag=btag, bufs=1,
                                     name=f"p_b{b}")
                    p_as[b], p_bs[b] = p_a, p_b
                    pt_dram = dpool.tile([Tt, P, S], F8E4, tag="pt_dram",
                                         name=f"ptd{b}")
                    eit_dram = dpool.tile([P, Ct, S], BF16, tag="eit_dram",
                                          name=f"eitd{b}")
                    pt_drams[b], eit_drams[b] = pt_dram, eit_dram
                    rzc = sp.tile([P, Tt], FP32, tag="rzc", name=f"rzc{b}")
                    rzd = sp.tile([P, St], FP32, tag="rzd", name=f"rzd{b}")
                    rzcs[b], rzds[b] = rzc, rzd
                    zd = sp.tile([P, St], FP32, tag="zd", bufs=2, name="zd")

                    # pre-issued first dispatch xk loads (filled at t==1: at
                    # t==0 they'd push the first router loads off the pipe)
                    xk_pre = []

                    def emit_xk_pre():
                        for k in range(6):
                            xk = p3.tile([P, 512], BF16, tag="xk", bufs=6,
                                         name="xk")
                            nc.sync.dma_start(
                                xk[:],
                                xb[b].rearrange("(k p) c -> p k c", p=P)[
                                    :, k, 0:512
                                ],
                            )
                            xk_pre.append(xk)

                    def load_xbt(t):
                        xbt_t = p3.tile([P, Ct, P], BF16, tag="xbt_t", bufs=3,
                                        name="xbt_t")
                        nc.sync.dma_start(
                            xbt_t[:],
                            xbt[b].rearrange("(c p) t -> p c t", p=P)[
                                :, :, t * P : (t + 1) * P
                            ],
                        )
                        return xbt_t

                    def router_tail(t, gps):
                        zc4 = sp.tile([P, 4], FP32, tag="zc4", bufs=2, name="zc4")
                        for n in range(4):
                            dst = (
                                p_a[:, t, n * 512 : (n + 1) * 512]
                                if n < 2
                                else p_b[:, t, (n - 2) * 512 : (n - 1) * 512]
                            )
                            nc.scalar.activation(
                                dst,
                                gps[n][:],
                                AF.Exp,
                                accum_out=zc4[:, n : n + 1],
                            )
                        zc1 = sp.tile([P, 1], FP32, tag="zc1", bufs=2, name="zc1")
                        nc.vector.tensor_reduce(zc1[:], zc4[:], mybir.AxisListType.X, AX.add)
                        nc.vector.reciprocal(rzc[:, t : t + 1], zc1[:])

                        # transpose chunk t in two half-slabs (smaller
                        # staging; DMA engines overlap PE on t+1)
                        for sh in range(2):
                            src = p_a if sh == 0 else p_b
                            ptc = p3.tile([P, St // 2, P], BF16, tag="ptc",
                                          bufs=2, name="ptc")
                            nc.scalar.dma_start_transpose(
                                ptc[:], src[:, t, :]
                            )
                            zds = zd[:, sh * 8 : (sh + 1) * 8]
                            if t == 0:
                                nc.vector.tensor_reduce(
                                    zds, ptc[:], mybir.AxisListType.X, AX.add
                                )
                            else:
                                zdt = sp.tile([P, St // 2], FP32, tag="zdt",
                                              bufs=2, name="zdt")
                                nc.vector.tensor_reduce(
                                    zdt[:], ptc[:], mybir.AxisListType.X, AX.add
                                )
                                nc.vector.tensor_add(zds, zds, zdt[:])
                            nc.gpsimd.dma_start(
                                pt_dram[t, :, sh * 1024 : (sh + 1) * 1024],
                                ptc[:],
                            )

                    tstart = 0
                    if rep == 0 and b == 0:
                        # wrt is still streaming in from HBM: accumulate the
                        # first two token-tiles chunk-by-chunk across all 8
                        # PSUM banks so PE works while the 8MB load lands
                        tstart = 2
                        xbt01 = [load_xbt(0), load_xbt(1)]
                        # preload t=2,3's xbt into ht's idle slots so the
                        # t-loop doesn't stall on slot-blocked loads after
                        # the c-major block releases t0/t1
                        xbt23 = []
                        for tt in (2, 3):
                            xt = p2.tile([P, Ct, P], BF16, tag="ht", bufs=4,
                                         name="xbt_t")
                            nc.sync.dma_start(
                                xt[:],
                                xbt[b].rearrange("(c p) t -> p c t", p=P)[
                                    :, :, tt * P : (tt + 1) * P
                                ],
                            )
                            xbt23.append(xt)
                        emit_xk_pre()
                        gps01 = [
                            [pp.tile([P, 512], FP32,
                                     tag=("pa" if tt == 0 else "pb"),
                                     name=f"gps{tt}_{n}") for n in range(4)]
                            for tt in range(2)
                        ]
                        for c in range(Ct):
                            for tt in range(2):
                                for n in range(4):
                                    nc.tensor.matmul(
                                        gps01[tt][n][:],
                                        xbt01[tt][:, c, :],
                                        wrt_sb[:, c, n * 512 : (n + 1) * 512],
                                        start=(c == 0),
                                        stop=(c == Ct - 1),
                                    )
                        router_tail(0, gps01[0])
                        router_tail(1, gps01[1])

                    for t in range(tstart, Tt):
                        if t == 1:
                            emit_xk_pre()
                        if tstart == 2 and t in (2, 3):
                            xbt_t = xbt23[t - 2]
                        else:
                            xbt_t = load_xbt(t)
                        gps = [pp.tile([P, 512], FP32, tag="pa", name=f"gps{n}")
                               for n in range(4)]
                        for c in range(Ct):
                            for n in range(4):
                                nc.tensor.matmul(
                                    gps[n][:],
                                    xbt_t[:, c, :],
                                    wrt_sb[:, c, n * 512 : (n + 1) * 512],
                                    start=(c == 0),
                                    stop=(c == Ct - 1),
                                )
                        router_tail(t, gps)

                    nc.vector.reciprocal(rzd[:], zd[:])
                    rzc2 = sp.tile([P, Tt], FP32, tag="rzc2", name=f"rzc2{b}")
                    nc.vector.tensor_scalar_mul(rzc2[:], rzc[:], 1.0 / EOSC)
                    rzc2s[b] = rzc2

                    # ---- D: dispatch eit = x^T @ P (unnormalized) ----
                    # nh outer: slot-halves complete in order, so expert 0's
                    # loads (prefetched below) start while nh=1 still runs.
                    for nh in range(2):
                        p_sb = p_as[b] if nh == 0 else p_bs[b]
                        if b == 1 and nh == 1:
                            pre_e0["eite"] = {}
                            for bb in range(BL):
                                eit_e = p2.tile([P, Ct, P], BF16, tag="eit_e",
                                                bufs=4, name=f"eite{bb}")
                                nc.sync.dma_start(
                                    eit_e[:],
                                    eit_drams[bb][:, :, 0:P],
                                )
                                pre_e0["eite"][bb] = eit_e
                            wgc = p2.tile([P, Ct, HQ], BF16, tag="wgc",
                                          bufs=4, name="wgc")
                            nc.sync.dma_start(
                                wgc[:],
                                wg[0].rearrange("(c p) h -> p c h", p=P)[:, :, 0:HQ],
                            )
                            wfcc = p2.tile([P, Ct, HQ], BF16, tag="wfcc",
                                           bufs=4, name="wfcc")
                            nc.gpsimd.dma_start(
                                wfcc[:],
                                wf[0].rearrange("(c p) h -> p c h", p=P)[:, :, 0:HQ],
                            )
                            pre_e0["w0"] = (wgc, wfcc)
                        for mh in range(2):
                            # pb first: pa banks are still draining the last
                            # router exp() when dispatch starts
                            dps = [pp.tile([P, 512], FP32,
                                           tag=("pb" if i < 4 else "pa"),
                                           name=f"dps{i}")
                                   for i in range(8)]
                            for k in range(Tt):
                                if nh == 0 and mh == 0 and k < 6:
                                    xk = xk_pre[k]
                                else:
                                    xk = p3.tile([P, 512], BF16, tag="xk",
                                                 bufs=6, name="xk")
                                    nc.sync.dma_start(
                                        xk[:],
                                        xb[b].rearrange(
                                            "(k p) c -> p k c", p=P
                                        )[:, k, mh * 512 : (mh + 1) * 512],
                                    )
                                for m4 in range(4):
                                    for n2 in range(2):
                                        nc.tensor.matmul(
                                            dps[m4 * 2 + n2][:],
                                            xk[:, m4 * P : (m4 + 1) * P],
                                            p_sb[:, k,
                                                 n2 * 512 : (n2 + 1) * 512],
                                            start=(k == 0),
                                            stop=(k == Tt - 1),
                                        )
                            for i in range(8):
                                m = mh * 4 + i // 2
                                n = nh * 2 + i % 2
                                est = p2.tile([P, 512], BF16, tag="wfcc",
                                              bufs=4, name="est")
                                _copy(i, est[:], dps[i][:])
                                nc.gpsimd.dma_start(
                                    eit_dram[:, m, n * 512 : (n + 1) * 512],
                                    est[:],
                                )

                # ---- M: per-expert GLU MLP, software-pipelined: expert e's
                # gg/hh runs on PE while expert e-1's h transposes on the DMA
                # engines; e-1's eo matmuls then fill what would be the
                # transpose stall. eo -> one fp8 tile resident in P-A's arena.
                eo_t = cpool.tile([P, BL, St, C], F8E4, tag="bigY", bufs=1,
                                  name="eo_t")
                if rep + 1 < nreps:
                    wrt_next = cpool.tile([P, Ct, S], BF16, tag=btag, bufs=1,
                                          name="wrt_sb")
                else:
                    wrt_next = None

                def emit_prefetch(e, into):
                    into["eite"] = {}

                    def _eite(bb):
                        eit_e = p2.tile([P, Ct, P], BF16, tag="eit_e",
                                        bufs=4, name=f"eite{bb}")
                        nc.sync.dma_start(
                            eit_e[:],
                            eit_drams[bb][:, :, e * P : (e + 1) * P],
                        )
                        into["eite"][bb] = eit_e

                    _eite(0)
                    wgc = p2.tile([P, Ct, HQ], BF16, tag="wgc", bufs=4,
                                  name="wgc")
                    nc.sync.dma_start(
                        wgc[:],
                        wg[e].rearrange("(c p) h -> p c h", p=P)[:, :, 0:HQ],
                    )
                    wfcc = p2.tile([P, Ct, HQ], BF16, tag="wfcc", bufs=4,
                                   name="wfcc")
                    nc.sync.dma_start(
                        wfcc[:],
                        wf[e].rearrange("(c p) h -> p c h", p=P)[:, :, 0:HQ],
                    )
                    _eite(1)
                    into["w0"] = (wgc, wfcc)

                def issue_wpc_chunk(e, kh):
                    # w_proj half-chunk (2 k-tiles) for expert e's eo
                    w = p2.tile([P, 2, C], BF16, tag="wpc", bufs=5,
                                name="wpc")
                    nc.gpsimd.dma_start(
                        w[:],
                        wp[e].rearrange("(k p) c -> p k c", p=P)[
                            :, kh * 2 : (kh + 1) * 2, :
                        ],
                    )
                    return w

                def issue_wpc(e):
                    return [issue_wpc_chunk(e, kh) for kh in range(8)]

                def emit_gghh(e, pre):
                    if wrt_next is not None and 3 <= e < 3 + Ct:
                        emit_wrt_chunk(wrt_next, e - 3)
                    eites = pre["eite"]
                    hs = {
                        b: p2.tile([P, H], BF16, tag=f"hs{b}", bufs=1, name=f"hsb{b}")
                        for b in range(BL)
                    }
                    for hc in range(H // HQ):
                        if hc == 0:
                            wgc, wfcc = pre["w0"]
                        else:
                            wgc = p2.tile([P, Ct, HQ], BF16, tag="wgc",
                                          bufs=4, name="wgc")
                            nc.sync.dma_start(
                                wgc[:],
                                wg[e].rearrange("(c p) h -> p c h", p=P)[
                                    :, :, hc * HQ : (hc + 1) * HQ
                                ],
                            )
                            wfcc = p2.tile([P, Ct, HQ], BF16, tag="wfcc",
                                           bufs=4, name="wfcc")
                            nc.sync.dma_start(
                                wfcc[:],
                                wf[e].rearrange("(c p) h -> p c h", p=P)[
                                    :, :, hc * HQ : (hc + 1) * HQ
                                ],
                            )
                        for b in range(BL):
                            gg = pp.tile([P, HQ], FP32, tag="pa", name=f"gg{b}")
                            hh = pp.tile([P, HQ], FP32, tag="pa", name=f"hh{b}")
                            for c in range(Ct):
                                # same stationary operand back-to-back: lets
                                # codegen/hw skip the redundant LDWEIGHTS
                                nc.tensor.matmul(
                                    gg[:], eites[b][:, c, :], wgc[:, c, :],
                                    start=(c == 0), stop=(c == Ct - 1),
                                )
                                nc.tensor.matmul(
                                    hh[:], eites[b][:, c, :], wfcc[:, c, :],
                                    start=(c == 0), stop=(c == Ct - 1),
                                )
                            sg = p3.tile([P, HQ], BF16, tag="sg", bufs=1,
                                         name="sg")
                            nc.scalar.activation(
                                sg[:], gg[:], AF.Silu,
                                scale=rzds[b][:, e : e + 1],
                            )
                            nc.vector.scalar_tensor_tensor(
                                hs[b][:, hc * HQ : (hc + 1) * HQ],
                                hh[:], rzds[b][:, e : e + 1], sg[:],
                                AX.mult, AX.mult,
                            )
                    hts = {}
                    for b in range(BL):
                        ht = p2.tile([P, Ht, P], BF16, tag="ht", bufs=4, name=f"htb{b}")
                        nc.scalar.dma_start_transpose(ht[:], hs[b][:])
                        hts[b] = ht
                    return hts

                def emit_eo(e, hts, wpcs, wpc_pre_e=None, wpcs_out=None):
                    # cc innermost: both C-halves reuse the same stationary
                    # ht chunk (halves the LDWEIGHTS count). The next
                    # expert's w_proj chunks are issued here — eo's window
                    # is when the DMA pipe is otherwise idle.
                    eops = {
                        b: [pp.tile([P, 512], FP32, tag="pb",
                                    name=f"eop{b}_{cc}") for cc in range(2)]
                        for b in range(BL)
                    }
                    for kh in range(8):
                        if wpc_pre_e is not None:
                            wpcs_out[wpc_pre_e].append(
                                issue_wpc_chunk(wpc_pre_e, kh)
                            )
                        wpc = wpcs[kh]
                        for b in range(BL):
                            for k in range(2):
                                for cc in range(2):
                                    nc.tensor.matmul(
                                        eops[b][cc][:],
                                        hts[b][:, kh * 2 + k, :],
                                        wpc[:, k, cc * 512 : (cc + 1) * 512],
                                        start=(kh == 0 and k == 0),
                                        stop=(kh == 7 and k == 1),
                                    )
                    for b in range(BL):
                        for cc in range(2):
                            nc.vector.tensor_scalar_mul(
                                eo_t[:, b, e, cc * 512 : (cc + 1) * 512],
                                eops[b][cc][:], EOSC,
                            )

                prev = None
                pre = pre_e0
                wpcs_store = {}
                for e in range(E):
                    hts, pre_next = emit_gghh(e, pre), {}
                    if e == 0:
                        wpcs_store[0] = issue_wpc(0)
                    if e + 1 < E:
                        emit_prefetch(e + 1, pre_next)
                    if prev is not None:
                        nxt = e if e < E else None
                        wpcs_store[nxt] = []
                        emit_eo(prev[0], prev[1], wpcs_store.pop(prev[0]),
                                wpc_pre_e=nxt, wpcs_out=wpcs_store)
                    prev = (e, hts)
                    pre = pre_next
                emit_eo(prev[0], prev[1], wpcs_store.pop(prev[0]))

                # ---- C: combine y = (P^T^T @ eo) * rzc ----
                for b in range(BL):
                    for t in range(Tt):
                        ptr = p3.tile([P, St, P], F8E4, tag="ptr", bufs=3,
                                      name="ptr")
                        nc.sync.dma_start(ptr[:], pt_drams[b][t])
                        ypss = [pp.tile([P, 512], FP32,
                                        tag=("pa" if cc == 0 else "pb"),
                                        name=f"yps{cc}") for cc in range(2)]
                        for e in range(0, St, 2):
                            for cc in range(2):
                                nc.tensor.matmul(
                                    ypss[cc][:],
                                    ptr[:, e : e + 2, :],
                                    eo_t[:, b, e : e + 2,
                                         cc * 512 : (cc + 1) * 512],
                                    start=(e == 0),
                                    stop=(e == St - 2),
                                    perf_mode=DR,
                                )
                        for cc in range(2):
                            # wgc's arena is idle during combine: borrow
                            # its slots (same tag => serial reuse with deps)
                            ysb = p2.tile([P, 512], FP32, tag="wgc", bufs=4,
                                          name="ysb")
                            nc.vector.tensor_scalar_mul(
                                ysb[:], ypss[cc][:], rzc2s[b][:, t : t + 1]
                            )
                            # y stores split across both HWDGE rings: the
                            # store-completion -> ysb-slot -> DVE-scale ->
                            # PSUM-release chain paces the combine, and one
                            # ring's completion latency is the bottleneck.
                            # (gpsimd's Q7 would back up under 64 stores.)
                            eng = nc.scalar if cc == 0 else nc.sync
                            eng.dma_start(
                                y[b, t * P : (t + 1) * P,
                                  cc * 512 : (cc + 1) * 512],
                                ysb[:],
                            )
    if split_waits:
        _split_multi_waits(nc)
    return nc


def make_in_maps(x, w_router_gate, w_fc, w_gate, w_proj):
    bf16 = ml_dtypes.bfloat16
    wrt_np = np.ascontiguousarray(w_router_gate.reshape(S, C).T).astype(bf16)
    wg_np = w_gate.astype(bf16)
    wf_np = w_fc.astype(bf16)
    wp_np = w_proj.astype(bf16)

    in_maps = []
    for c in range(NCORES):
        xc = x[c * BL : (c + 1) * BL]
        xb_np = xc.astype(bf16)
        xbt_np = np.ascontiguousarray(xb_np.transpose(0, 2, 1))
        in_maps.append(
            {"xb": xb_np, "xbt": xbt_np, "wrt": wrt_np,
             "wg": wg_np, "wf": wf_np, "wp": wp_np}
        )
    return in_maps


def kernel(x, w_router_gate, w_fc, w_gate, w_proj):
    in_maps = make_in_maps(x, w_router_gate, w_fc, w_gate, w_proj)

    from concourse.bass_utils import run_bass_kernel_spmd

    nc = build_nc()
    res = None
    last_err = None
    for attempt in range(4):
        try:
            res = run_bass_kernel_spmd(nc, in_maps, core_ids=list(range(NCORES)))
            break
        except Exception as e:  # transient NRT_EXEC_UNIT_UNRECOVERABLE on first exec
            last_err = e
            import time as _time

            _time.sleep(5)
    if res is None:
        raise last_err
    y = np.concatenate(
        [res.results[c]["y"] for c in range(NCORES)], axis=0
    ).astype(np.float32)
    return y


if __name__ == "__main__":
    print("built", build_nc())


# revision 51
# speedup vs baseline: 1.0866x; 1.0106x over previous
"""Soft-MoE forward on 8 TRN2 NeuronCores — v7 (TimelineSim-guided pipelining).

Data-parallel over batch (B=16 -> 2 per core). Matmuls in bf16 except the
combine, which runs fp8e4 DoubleRow (half the instructions at 2x rate).
Full fp8 breaks the 2e-2 accuracy gate (quantization noise ~3.6%/tensor
does not average through random-sign contractions; measured 6.8e-2
all-fp8, 3.5e-2 for router/dispatch-fp8) but combine-only fp8 lands at
1.36e-2: PT is cast bf16->fp8 for free in the gpsimd store DMA, and eo is
scaled by 2^13 into fp8 range (descale folded into the rzc factor).

v10 structural changes vs v2 (each validated against concourse TimelineSim;
single-rep sim 1596us -> 1303us; HW-validated rel err 1.363581e-2):
- P split into two 32KB arenas (slot halves A/B); eo is one [P,BL,St,C]
  fp8 tile reusing arena A's slot. Arena B and the wrt arena alternate
  roles per rep ("bigX"/"bigZ") so the NEXT rep's router weights (8MB)
  prefetch during the expert phase instead of stalling the rep boundary.
- w_proj streams as 8 half-chunks/expert (bufs=5) issued from inside the
  PREVIOUS expert's eo — the window where the DMA pipe is otherwise
  idle — so they neither stall eo nor crowd out wg/wf chunk loads.
- wg/wf stream in [P,Ct,256] chunks (8/expert, bufs=4) on the sync ring
  only: DMA doorbells that wait on buffer slots must not share a
  sequencer with compute (ACT) — head-of-line blocking of the silu
  delays PSUM release and stalls PE. bufs=4 (not 3) was the binding
  constraint for gghh streaming (-49us).
- fp8 limits (numpy-emulated, calibrates to HW within 0.02e-2): w_proj
  fp8 = 3.2e-2 (fails), h-only fp8 = 1.62e-2 (passes but matmul
  operands must share dtype, so no eo DoubleRow). eo stays bf16.
- ht bufs=4 for the same reason: the h-transpose doorbell on the ACT
  ring must never wait on a slot.
- Router t=0/t=1 of the first batch accumulate c-chunk-major across all
  8 PSUM banks so PE works while the 8MB wrt load is still landing.
- First 5 dispatch xk loads pre-issued at router t==1 (the list
  scheduler otherwise parks them behind the next batch's router loads,
  which head-of-line block the SP ring for ~7us).
- Dispatch PSUM tags flipped (pb first): pa banks still drain the last
  router exp() when dispatch starts.
- y stores on the scalar HWDGE ring (gpsimd Q7 backs up under 64
  stores); last tiles split across both rings to shorten the drain.
- zd accumulated with running DVE adds (no [P,St,Tt] zdall buffer).
- Phase-disjoint tag sharing: tiles of different shapes may share a tag
  (serial slot reuse, deps auto-inserted). Combine's ysb staging lives
  in wgc's arena and its ptr tiles in wpc's (both idle during combine,
  ptr gains 5-deep prefetch), dispatch's est in wfcc's (idle during
  dispatch), startup xbt t=2/3 in ht's (idle at rep start) — freeing
  ~15KB of dedicated arenas to fund xk=6/eite=4/xbt_t=3 depth.

DO NOT change dynamic_dma_scratch_size: a non-default SWDGE carveout
(12288) produced NaN output on hardware (descriptor-ring corruption).
"""

import numpy as np
import ml_dtypes

import concourse.bass as bass
import concourse.tile as tile
from concourse import mybir

B, T, C, E, H = 16, 2048, 1024, 16, 2048
CAP = T // E  # 128
S = E * CAP  # 2048 slots
P = 128
NCORES = 8
BL = B // NCORES  # 2

Tt = T // P   # 16
Ct = C // P   # 8
St = S // P   # 16
Ht = H // P   # 16

FP32 = mybir.dt.float32
BF16 = mybir.dt.bfloat16
F8E4 = mybir.dt.float8e4
DR = mybir.MatmulPerfMode.DoubleRow
EOSC = 2.0 ** 13  # eo fp8 storage scale; folded out via rzc
AX = mybir.AluOpType
AF = mybir.ActivationFunctionType


def _split_multi_waits(nc):
    """This walrus build accepts only ONE sync wait per instruction; Tile's
    wait-assignment can emit several. Move extra waits onto single-wait nops
    inserted just before the instruction on the same engine."""
    import bass_rust

    nid = 0
    for f in nc.m.functions:
        for bb in f.blocks:
            out = []
            changed = False
            for inst in bb.instructions:
                si = inst.sync_info
                waits = list(si.on_wait) if si and si.on_wait else []
                if len(waits) > 1:
                    changed = True
                    for w in waits[:-1]:
                        nop = mybir.InstNoOp(name=f"TW-{nid}", ins=[], outs=[])
                        nid += 1
                        nop.engine = inst.engine
                        nop.sync_info = bass_rust.SyncInfo(on_wait=[w], on_update=[])
                        out.append(nop)
                    si.on_wait = waits[-1:]
                out.append(inst)
            if changed:
                bb.instructions = out


def build_nc(loops=BL, split_waits=True):
    assert loops % BL == 0
    nreps = loops // BL
    nc = bass.Bass(trn_type="TRN2")

    xb = nc.dram_tensor("xb", [BL, T, C], BF16, kind="ExternalInput")
    xbt = nc.dram_tensor("xbt", [BL, C, T], BF16, kind="ExternalInput")
    wrt = nc.dram_tensor("wrt", [C, S], BF16, kind="ExternalInput")
    wg = nc.dram_tensor("wg", [E, C, H], BF16, kind="ExternalInput")
    wf = nc.dram_tensor("wf", [E, C, H], BF16, kind="ExternalInput")
    wp = nc.dram_tensor("wp", [E, H, C], BF16, kind="ExternalInput")
    y = nc.dram_tensor("y", [BL, T, C], FP32, kind="ExternalOutput")

    HQ = 256  # wg/wf h-chunk width

    with tile.TileContext(nc) as tc:
        with (
            tc.tile_pool(name="dram", bufs=2, space="DRAM") as dpool,
            tc.tile_pool(name="big", bufs=1) as cpool,
            tc.tile_pool(name="st3", bufs=3) as p3,
            tc.tile_pool(name="st2", bufs=2) as p2,
            tc.tile_pool(name="stat", bufs=2) as sp,
            tc.tile_pool(name="psum", bufs=4, space="PSUM") as pp,
        ):
            # round-robin PSUM->SBUF copy engine to avoid single-queue tails
            def _copy(i, out, in_):
                # gpsimd cannot read PSUM (walrus birverifier)
                if i % 2 == 0:
                    nc.scalar.copy(out, in_)
                else:
                    nc.vector.tensor_copy(out, in_)

            def emit_wrt_chunk(wtile, c):
                eng = nc.scalar if c % 2 == 0 else nc.gpsimd
                eng.dma_start(
                    wtile[:, c], wrt.rearrange("(c p) s -> p c s", p=P)[:, c]
                )

            # rep-parity arena roles: "bigY" always holds P-half-A then eo;
            # "bigX"/"bigZ" alternate between wrt and P-half-B so the next
            # rep's wrt prefetches into the arena freed by this rep's P-B.
            wrt_next = cpool.tile([P, Ct, S], BF16, tag="bigX", bufs=1,
                                  name="wrt_sb")
            for c in range(Ct):
                emit_wrt_chunk(wrt_next, c)

            for rep in range(nreps):
                wrt_sb = wrt_next
                btag = "bigZ" if rep % 2 == 0 else "bigX"

                pt_drams, rzcs, rzds = {}, {}, {}
                rzc2s = {}
                eit_drams = {}
                p_as, p_bs = {}, {}
                pre_e0 = {}
                for b in range(BL):
                    # ---- R+T: router logits, exp, transpose chunks ----
                    p_a = cpool.tile([P, Tt, S // 2], BF16, tag="bigY", bufs=1,
                                     name=f"p_a{b}")
                    p_b = cpool.tile([P, Tt, S // 2], BF16, tag=btag, bufs=1,
                                     name=f"p_b{b}")
                    p_as[b], p_bs[b] = p_a, p_b
                    pt_dram = dpool.tile([Tt, P, S], F8E4, tag="pt_dram",
                                         name=f"ptd{b}")
                    eit_dram = dpool.tile([P, Ct, S], BF16, tag="eit_dram",
                                          name=f"eitd{b}")
                    pt_drams[b], eit_drams[b] = pt_dram, eit_dram
                    rzc = sp.tile([P, Tt], FP32, tag="rzc", name=f"rzc{b}")
                    rzd = sp.tile([P, St], FP32, tag="rzd", name=f"rzd{b}")
                    rzcs[b], rzds[b] = rzc, rzd
                    zd = sp.tile([P, St], FP32, tag="zd", bufs=2, name="zd")

                    # pre-issued first dispatch xk loads (filled at t==1: at
                    # t==0 they'd push the first router loads off the pipe)
                    xk_pre = []

                    def emit_xk_pre():
                        for k in range(6):
                            xk = p3.tile([P, 512], BF16, tag="xk", bufs=6,
                                         name="xk")
                            nc.sync.dma_start(
                                xk[:],
                                xb[b].rearrange("(k p) c -> p k c", p=P)[
                                    :, k, 0:512
                                ],
                            )
                            xk_pre.append(xk)

                    def load_xbt(t):
                        xbt_t = p3.tile([P, Ct, P], BF16, tag="xbt_t", bufs=3,
                                        name="xbt_t")
                        nc.sync.dma_start(
                            xbt_t[:],
                            xbt[b].rearrange("(c p) t -> p c t", p=P)[
                                :, :, t * P : (t + 1) * P
                            ],
                        )
                        return xbt_t

                    def router_tail(t, gps):
                        zc4 = sp.tile([P, 4], FP32, tag="zc4", bufs=2, name="zc4")
                        for n in range(4):
                            dst = (
                                p_a[:, t, n * 512 : (n + 1) * 512]
                                if n < 2
                                else p_b[:, t, (n - 2) * 512 : (n - 1) * 512]
                            )
                            nc.scalar.activation(
                                dst,
                                gps[n][:],
                                AF.Exp,
                                accum_out=zc4[:, n : n + 1],
                            )
                        zc1 = sp.tile([P, 1], FP32, tag="zc1", bufs=2, name="zc1")
                        nc.vector.tensor_reduce(zc1[:], zc4[:], mybir.AxisListType.X, AX.add)
                        nc.vector.reciprocal(rzc[:, t : t + 1], zc1[:])

                        # transpose chunk t in two half-slabs (smaller
                        # staging; DMA engines overlap PE on t+1)
                        for sh in range(2):
                            src = p_a if sh == 0 else p_b
                            ptc = p3.tile([P, St // 2, P], BF16, tag="ptc",
                                          bufs=2, name="ptc")
                            nc.scalar.dma_start_transpose(
                                ptc[:], src[:, t, :]
                            )
                            zds = zd[:, sh * 8 : (sh + 1) * 8]
                            if t == 0:
                                nc.vector.tensor_reduce(
                                    zds, ptc[:], mybir.AxisListType.X, AX.add
                                )
                            else:
                                zdt = sp.tile([P, St // 2], FP32, tag="zdt",
                                              bufs=2, name="zdt")
                                nc.vector.tensor_reduce(
                                    zdt[:], ptc[:], mybir.AxisListType.X, AX.add
                                )
                                nc.vector.tensor_add(zds, zds, zdt[:])
                            nc.gpsimd.dma_start(
                                pt_dram[t, :, sh * 1024 : (sh + 1) * 1024],
                                ptc[:],
                            )

                    tstart = 0
                    if rep == 0 and b == 0:
                        # wrt is still streaming in from HBM: accumulate the
                        # first two token-tiles chunk-by-chunk across all 8
                        # PSUM banks so PE works while the 8MB load lands
                        tstart = 2
                        xbt01 = [load_xbt(0), load_xbt(1)]
                        # preload t=2,3's xbt into ht's idle slots so the
                        # t-loop doesn't stall on slot-blocked loads after
                        # the c-major block releases t0/t1
                        xbt23 = []
                        for tt in (2, 3):
                            xt = p2.tile([P, Ct, P], BF16, tag="ht", bufs=4,
                                         name="xbt_t")
                            nc.sync.dma_start(
                                xt[:],
                                xbt[b].rearrange("(c p) t -> p c t", p=P)[
                                    :, :, tt * P : (tt + 1) * P
                                ],
                            )
                            xbt23.append(xt)
                        emit_xk_pre()
                        gps01 = [
                            [pp.tile([P, 512], FP32,
                                     tag=("pa" if tt == 0 else "pb"),
                                     name=f"gps{tt}_{n}") for n in range(4)]
                            for tt in range(2)
                        ]
                        for c in range(Ct):
                            for tt in range(2):
                                for n in range(4):
                                    nc.tensor.matmul(
                                        gps01[tt][n][:],
                                        xbt01[tt][:, c, :],
                                        wrt_sb[:, c, n * 512 : (n + 1) * 512],
                                        start=(c == 0),
                                        stop=(c == Ct - 1),
                                    )
                        router_tail(0, gps01[0])
                        router_tail(1, gps01[1])

                    for t in range(tstart, Tt):
                        if t == 1:
                            emit_xk_pre()
                        if tstart == 2 and t in (2, 3):
                            xbt_t = xbt23[t - 2]
                        else:
                            xbt_t = load_xbt(t)
                        gps = [pp.tile([P, 512], FP32, tag="pa", name=f"gps{n}")
                               for n in range(4)]
                        for c in range(Ct):
                            for n in range(4):
                                nc.tensor.matmul(
                                    gps[n][:],
                                    xbt_t[:, c, :],
                                    wrt_sb[:, c, n * 512 : (n + 1) * 512],
                                    start=(c == 0),
                                    stop=(c == Ct - 1),
                                )
                        router_tail(t, gps)

                    nc.vector.reciprocal(rzd[:], zd[:])
                    rzc2 = sp.tile([P, Tt], FP32, tag="rzc2", name=f"rzc2{b}")
                    nc.vector.tensor_scalar_mul(rzc2[:], rzc[:], 1.0 / EOSC)
                    rzc2s[b] = rzc2

                    # ---- D: dispatch eit = x^T @ P (unnormalized) ----
                    # nh outer: slot-halves complete in order, so expert 0's
                    # loads (prefetched below) start while nh=1 still runs.
                    for nh in range(2):
                        p_sb = p_as[b] if nh == 0 else p_bs[b]
                        if b == 1 and nh == 1:
                            pre_e0["eite"] = {}
                            for bb in range(BL):
                                eit_e = p2.tile([P, Ct, P], BF16, tag="eit_e",
                                                bufs=4, name=f"eite{bb}")
                                nc.sync.dma_start(
                                    eit_e[:],
                                    eit_drams[bb][:, :, 0:P],
                                )
                                pre_e0["eite"][bb] = eit_e
                            wgc = p2.tile([P, Ct, HQ], BF16, tag="wgc",
                                          bufs=4, name="wgc")
                            nc.sync.dma_start(
                                wgc[:],
                                wg[0].rearrange("(c p) h -> p c h", p=P)[:, :, 0:HQ],
                            )
                            wfcc = p2.tile([P, Ct, HQ], BF16, tag="wfcc",
                                           bufs=4, name="wfcc")
                            nc.gpsimd.dma_start(
                                wfcc[:],
                                wf[0].rearrange("(c p) h -> p c h", p=P)[:, :, 0:HQ],
                            )
                            pre_e0["w0"] = (wgc, wfcc)
                        for mh in range(2):
                            # pb first: pa banks are still draining the last
                            # router exp() when dispatch starts
                            dps = [pp.tile([P, 512], FP32,
                                           tag=("pb" if i < 4 else "pa"),
                                           name=f"dps{i}")
                                   for i in range(8)]
                            for k in range(Tt):
                                if nh == 0 and mh == 0 and k < 6:
                                    xk = xk_pre[k]
                                else:
                                    xk = p3.tile([P, 512], BF16, tag="xk",
                                                 bufs=6, name="xk")
                                    nc.sync.dma_start(
                                        xk[:],
                                        xb[b].rearrange(
                                            "(k p) c -> p k c", p=P
                                        )[:, k, mh * 512 : (mh + 1) * 512],
                                    )
                                for m4 in range(4):
                                    for n2 in range(2):
                                        nc.tensor.matmul(
                                            dps[m4 * 2 + n2][:],
                                            xk[:, m4 * P : (m4 + 1) * P],
                                            p_sb[:, k,
                                                 n2 * 512 : (n2 + 1) * 512],
                                            start=(k == 0),
                                            stop=(k == Tt - 1),
                                        )
                            for i in range(8):
                                m = mh * 4 + i // 2
                                n = nh * 2 + i % 2
                                est = p2.tile([P, 512], BF16, tag="wfcc",
                                              bufs=4, name="est")
                                _copy(i, est[:], dps[i][:])
                                nc.gpsimd.dma_start(
                                    eit_dram[:, m, n * 512 : (n + 1) * 512],
                                    est[:],
                                )

                # ---- M: per-expert GLU MLP, software-pipelined: expert e's
                # gg/hh runs on PE while expert e-1's h transposes on the DMA
                # engines; e-1's eo matmuls then fill what would be the
                # transpose stall. eo -> one fp8 tile resident in P-A's arena.
                eo_t = cpool.tile([P, BL, St, C], F8E4, tag="bigY", bufs=1,
                                  name="eo_t")
                if rep + 1 < nreps:
                    wrt_next = cpool.tile([P, Ct, S], BF16, tag=btag, bufs=1,
                                          name="wrt_sb")
                else:
                    wrt_next = None

                def emit_prefetch(e, into):
                    into["eite"] = {}

                    def _eite(bb):
                        eit_e = p2.tile([P, Ct, P], BF16, tag="eit_e",
                                        bufs=4, name=f"eite{bb}")
                        nc.sync.dma_start(
                            eit_e[:],
                            eit_drams[bb][:, :, e * P : (e + 1) * P],
                        )
                        into["eite"][bb] = eit_e

                    _eite(0)
                    wgc = p2.tile([P, Ct, HQ], BF16, tag="wgc", bufs=4,
                                  name="wgc")
                    nc.sync.dma_start(
                        wgc[:],
                        wg[e].rearrange("(c p) h -> p c h", p=P)[:, :, 0:HQ],
                    )
                    wfcc = p2.tile([P, Ct, HQ], BF16, tag="wfcc", bufs=4,
                                   name="wfcc")
                    nc.sync.dma_start(
                        wfcc[:],
                        wf[e].rearrange("(c p) h -> p c h", p=P)[:, :, 0:HQ],
                    )
                    _eite(1)
                    into["w0"] = (wgc, wfcc)

                def issue_wpc_chunk(e, kh):
                    # w_proj half-chunk (2 k-tiles) for expert e's eo
                    w = p2.tile([P, 2, C], BF16, tag="wpc", bufs=5,
                                name="wpc")
                    nc.gpsimd.dma_start(
                        w[:],
                        wp[e].rearrange("(k p) c -> p k c", p=P)[
                            :, kh * 2 : (kh + 1) * 2, :
                        ],
                    )
                    return w

                def issue_wpc(e):
                    return [issue_wpc_chunk(e, kh) for kh in range(8)]

                def emit_gghh(e, pre):
                    if wrt_next is not None and 3 <= e < 3 + Ct:
                        emit_wrt_chunk(wrt_next, e - 3)
                    eites = pre["eite"]
                    hs = {
                        b: p2.tile([P, H], BF16, tag=f"hs{b}", bufs=1, name=f"hsb{b}")
                        for b in range(BL)
                    }
                    for hc in range(H // HQ):
                        if hc == 0:
                            wgc, wfcc = pre["w0"]
                        else:
                            wgc = p2.tile([P, Ct, HQ], BF16, tag="wgc",
                                          bufs=4, name="wgc")
                            nc.sync.dma_start(
                                wgc[:],
                                wg[e].rearrange("(c p) h -> p c h", p=P)[
                                    :, :, hc * HQ : (hc + 1) * HQ
                                ],
                            )
                            wfcc = p2.tile([P, Ct, HQ], BF16, tag="wfcc",
                                           bufs=4, name="wfcc")
                            nc.sync.dma_start(
                                wfcc[:],
                                wf[e].rearrange("(c p) h -> p c h", p=P)[
                                    :, :, hc * HQ : (hc + 1) * HQ
                                ],
                            )
                        for b in range(BL):
                            gg = pp.tile([P, HQ], FP32, tag="pa", name=f"gg{b}")
                            hh = pp.tile([P, HQ], FP32, tag="pa", name=f"hh{b}")
                            for c in range(Ct):
                                # same stationary operand back-to-back: lets
                                # codegen/hw skip the redundant LDWEIGHTS
                                nc.tensor.matmul(
                                    gg[:], eites[b][:, c, :], wgc[:, c, :],
                                    start=(c == 0), stop=(c == Ct - 1),
                                )
                                nc.tensor.matmul(
                                    hh[:], eites[b][:, c, :], wfcc[:, c, :],
                                    start=(c == 0), stop=(c == Ct - 1),
                                )
                            sg = p3.tile([P, HQ], BF16, tag="sg", bufs=1,
                                         name="sg")
                            nc.scalar.activation(
                                sg[:], gg[:], AF.Silu,
                                scale=rzds[b][:, e : e + 1],
                            )
                            nc.vector.scalar_tensor_tensor(
                                hs[b][:, hc * HQ : (hc + 1) * HQ],
                                hh[:], rzds[b][:, e : e + 1], sg[:],
                                AX.mult, AX.mult,
                            )
                    hts = {}
                    for b in range(BL):
                        ht = p2.tile([P, Ht, P], BF16, tag="ht", bufs=4, name=f"htb{b}")
                        nc.scalar.dma_start_transpose(ht[:], hs[b][:])
                        hts[b] = ht
                    return hts

                def emit_eo(e, hts, wpcs, wpc_pre_e=None, wpcs_out=None):
                    # cc innermost: both C-halves reuse the same stationary
                    # ht chunk (halves the LDWEIGHTS count). The next
                    # expert's w_proj chunks are issued here — eo's window
                    # is when the DMA pipe is otherwise idle.
                    eops = {
                        b: [pp.tile([P, 512], FP32, tag="pb",
                                    name=f"eop{b}_{cc}") for cc in range(2)]
                        for b in range(BL)
                    }
                    for kh in range(8):
                        if wpc_pre_e is not None:
                            wpcs_out[wpc_pre_e].append(
                                issue_wpc_chunk(wpc_pre_e, kh)
                            )
                        wpc = wpcs[kh]
                        for b in range(BL):
                            for k in range(2):
                                for cc in range(2):
                                    nc.tensor.matmul(
                                        eops[b][cc][:],
                                        hts[b][:, kh * 2 + k, :],
                                        wpc[:, k, cc * 512 : (cc + 1) * 512],
                                        start=(kh == 0 and k == 0),
                                        stop=(kh == 7 and k == 1),
                                    )
                    for b in range(BL):
                        for cc in range(2):
                            nc.vector.tensor_scalar_mul(
                                eo_t[:, b, e, cc * 512 : (cc + 1) * 512],
                                eops[b][cc][:], EOSC,
                            )

                prev = None
                pre = pre_e0
                wpcs_store = {}
                for e in range(E):
                    hts, pre_next = emit_gghh(e, pre), {}
                    if e == 0:
                        wpcs_store[0] = issue_wpc(0)
                    if e + 1 < E:
                        emit_prefetch(e + 1, pre_next)
                    if prev is not None:
                        nxt = e if e < E else None
                        wpcs_store[nxt] = []
                        emit_eo(prev[0], prev[1], wpcs_store.pop(prev[0]),
                                wpc_pre_e=nxt, wpcs_out=wpcs_store)
                    prev = (e, hts)
                    pre = pre_next
                emit_eo(prev[0], prev[1], wpcs_store.pop(prev[0]))

                # ---- C: combine y = (P^T^T @ eo) * rzc ----
                for b in range(BL):
                    for t in range(Tt):
                        # wpc's arena is idle during combine; its 2KB slots
                        # fit ptr exactly -> 5-deep prefetch + frees the old
                        # ptr arena
                        ptr = p2.tile([P, St, P], F8E4, tag="wpc", bufs=5,
                                      name="ptr")
                        nc.sync.dma_start(ptr[:], pt_drams[b][t])
                        ypss = [pp.tile([P, 512], FP32,
                                        tag=("pa" if cc == 0 else "pb"),
                                        name=f"yps{cc}") for cc in range(2)]
                        for e in range(0, St, 2):
                            for cc in range(2):
                                nc.tensor.matmul(
                                    ypss[cc][:],
                                    ptr[:, e : e + 2, :],
                                    eo_t[:, b, e : e + 2,
                                         cc * 512 : (cc + 1) * 512],
                                    start=(e == 0),
                                    stop=(e == St - 2),
                                    perf_mode=DR,
                                )
                        for cc in range(2):
                            # wgc's arena is idle during combine: borrow
                            # its slots (same tag => serial reuse with deps)
                            ysb = p2.tile([P, 512], FP32, tag="wgc", bufs=4,
                                          name="ysb")
                            nc.vector.tensor_scalar_mul(
                                ysb[:], ypss[cc][:], rzc2s[b][:, t : t + 1]
                            )
                            # y stores split across both HWDGE rings: the
                            # store-completion -> ysb-slot -> DVE-scale ->
                            # PSUM-release chain paces the combine, and one
                            # ring's completion latency is the bottleneck.
                            # (gpsimd's Q7 would back up under 64 stores.)
                            eng = nc.scalar if cc == 0 else nc.sync
                            eng.dma_start(
                                y[b, t * P : (t + 1) * P,
                                  cc * 512 : (cc + 1) * 512],
                                ysb[:],
                            )
    if split_waits:
        _split_multi_waits(nc)
    return nc


def make_in_maps(x, w_router_gate, w_fc, w_gate, w_proj):
    bf16 = ml_dtypes.bfloat16
    wrt_np = np.ascontiguousarray(w_router_gate.reshape(S, C).T).astype(bf16)
    wg_np = w_gate.astype(bf16)
    wf_np = w_fc.astype(bf16)
    wp_np = w_proj.astype(bf16)

    in_maps = []
    for c in range(NCORES):
        xc = x[c * BL : (c + 1) * BL]
        xb_np = xc.astype(bf16)
        xbt_np = np.ascontiguousarray(xb_np.transpose(0, 2, 1))
        in_maps.append(
            {"xb": xb_np, "xbt": xbt_np, "wrt": wrt_np,
             "wg": wg_np, "wf": wf_np, "wp": wp_np}
        )
    return in_maps


def kernel(x, w_router_gate, w_fc, w_gate, w_proj):
    in_maps = make_in_maps(x, w_router_gate, w_fc, w_gate, w_proj)

    from concourse.bass_utils import run_bass_kernel_spmd

    nc = build_nc()
    res = None
    last_err = None
    for attempt in range(4):
        try:
            res = run_bass_kernel_spmd(nc, in_maps, core_ids=list(range(NCORES)))
            break
        except Exception as e:  # transient NRT_EXEC_UNIT_UNRECOVERABLE on first exec
            last_err = e
            import time as _time

            _time.sleep(5)
    if res is None:
        raise last_err
    y = np.concatenate(
        [res.results[c]["y"] for c in range(NCORES)], axis=0
    ).astype(np.float32)
    return y


if __name__ == "__main__":
    print("built", build_nc())
